# revision 26
# baseline (speedup 1.0000x reference)
"""Trainium2 Bass kernel for nn_EquivariantBinaryClassificationSAGPoolScalar.

Algebraic reduction of the reference (per graph g):
  z=x@out_w, xs1=x@sc_w1+sc_b1, y2=x@sc_w2   (per-node scalars)
  W1=ea@dp_w1+dp_b1, W2=ea@dp_w2+dp_b2       (per-edge scalars)
  score1 = segment-mean over dst of xs1[src]*W1
  kept1 = top-512/graph (threshold = 512th largest), t1 = tanh(score1)
  m = kept1*(y2*t1 + sc_b2)
  score2 = segment-mean over dst of m[src]*W2 with count of (m[src] != 0)
  kept2 = top-256 among kept1 by score2, t2 = tanh(score2)
  out_g = sigmoid(sum_i z_i*(1 + kept1*t1*(1 + kept2*t2)) + out_b)

Node layout v2 (per core, NN=8192 nodes): node n lives at
  partition q(n) = (n//32) % 128   (= 32*(g%4) + hi,  hi=(n%1024)//32)
  column    C(n) = 32*(n//4096) + n%32  (= 32*(g//4) + lo, lo=n%32)
so graph g owns the [32*(g%4):+32, 32*(g//4):+32] block of every [128, 64]
per-node tile.  x loads with contiguous 8KB-per-partition DMAs; the bilinear
segment-sum lands into this layout with pure PE transposes (no scatter DMAs).

Edge slots: slot (p, s) holds edge e = 1024*(s//8) + 8*p + (s%8);
graph g owns slots s in [128g, 128g+128).  The gather runs as 4 ap_gather
calls per layer (double-buffered gout); call q covers slot columns with
s%128 in [32q, 32q+32), so the half-hh bilinear (slots s%128 in [64hh,+64))
starts after calls 2hh, 2hh+1 and overlaps the remaining gathers.
Host precomputes per-slot dst hi/lo (f32), gather indices (int16, in
bounce-table order), and layer-1 in-degree counts.
"""
import sys
import numpy as np

if "/opt/trn_rl_repo" not in sys.path:
    sys.path.insert(0, "/opt/trn_rl_repo")

import concourse.bass as bass
import concourse.bacc as bacc
import concourse.mybir as mybir
import concourse.tile as tile
from concourse.masks import make_identity
from concourse.bass_utils import run_bass_kernel_spmd

F32 = mybir.dt.float32
I32 = mybir.dt.int32
I16 = mybir.dt.int16
I8 = mybir.dt.int8
AL = mybir.AluOpType
ACTF = mybir.ActivationFunctionType

G = 8
NPG = 1024
NN = G * NPG
EPG = 16 * NPG
E = G * EPG
C = 256
EC = 48
K1 = NPG // 2
K2 = NPG // 4
NCOL = NN // 128
SLOTS = E // 128
HS = SLOTS // 2

Q1 = 1.0 - (K1 - 0.5) / (NPG - 1)      # k_adj = 510 -> out desc[511]
Q2 = 1.0 - (K2 - 1.5) / (K1 - 1)       # k_adj = 254 -> out desc[255]


def _ap(t, off_elems, free_dims):
    a = t[:]
    return bass.AP(a.tensor, a.offset + off_elems, [list(a.ap[0])] + free_dims)


def build_program(debug=False, reps=1, stage=99, loop=1, no_kth=False, no_gather=False):
    from contextlib import nullcontext
    nc = bacc.Bacc(None, target_bir_lowering=False, debug=False)

    x = nc.declare_dram_parameter("x", [NN, C], F32, isOutput=False)
    ea = nc.declare_dram_parameter("ea", [E, EC], F32, isOutput=False)
    lo_d = nc.declare_dram_parameter("lo_d", [128, SLOTS], F32, isOutput=False)
    hi_d = nc.declare_dram_parameter("hi_d", [128, SLOTS], F32, isOutput=False)
    gidx = nc.declare_dram_parameter("gidx", [128, SLOTS], I16, isOutput=False)
    cnt1_d = nc.declare_dram_parameter("cnt1_d", [128, NCOL], F32, isOutput=False)
    pr = {}
    for nm, shp in (("dp_w1", [EC, 1]), ("dp_b1", [1, 1]), ("sc_w1", [C, 1]),
                    ("sc_b1", [1, 1]), ("dp_w2", [EC, 1]), ("dp_b2", [1, 1]),
                    ("sc_w2", [C, 1]), ("sc_b2", [1, 1]), ("out_w", [C, 1]),
                    ("out_b", [1, 1]), ("iota32", [1, 32])):
        pr[nm] = nc.declare_dram_parameter(nm, shp, F32, isOutput=False)
    outp = nc.declare_dram_parameter("out", [G, 1], F32, isOutput=True)
    dbg = {}
    if debug:
        dbg["d_bs"] = nc.declare_dram_parameter("d_bs", [4, 40], F32, isOutput=True)
        for nm in ("d_proj", "d_num", "d_cnt", "d_score1", "d_kept1", "d_m",
                   "d_num2", "d_cnt2", "d_score2", "d_kept2", "d_compact1",
                   "d_compact2", "d_w"):
            shape = [128, SLOTS] if ("compact" in nm) else [128, NCOL]
            if nm == "d_proj":
                shape = [128, NCOL * 3]
            if nm == "d_w":
                shape = [128, SLOTS * 2]
            dbg[nm] = nc.declare_dram_parameter(nm, shape, F32, isOutput=True)

    bounce = nc.dram_tensor("bounce", [NN], F32)

    with tile.TileContext(nc) as tc:
        with (
            tc.tile_pool(name="const", bufs=1) as cpool,
            tc.tile_pool(name="node", bufs=1) as npool,
            tc.tile_pool(name="edge", bufs=1) as epool,
            tc.tile_pool(name="work", bufs=2) as wpool,
            tc.tile_pool(name="stage", bufs=4) as spool,
            tc.tile_pool(name="bil", bufs=2) as bpool,
            tc.tile_pool(name="ptr", bufs=6, space="PSUM") as pp_tr,
            tc.tile_pool(name="psb", bufs=2, space="PSUM") as ppsb,
        ):
            def ptile():
                pt = pp_tr.tile([128, 128], F32, tag="ptr")
                return pt

            # ---------------- constants ----------------
            ident = cpool.tile([128, 128], F32)
            make_identity(nc, ident[:])
            ones_r = cpool.tile([1, 128], F32)
            nc.vector.memset(ones_r[:], 1.0)

            blkmask = cpool.tile([128, 4], F32)
            nc.vector.memset(blkmask[:], 0.0)
            for j in range(4):
                nc.vector.memset(blkmask[32 * j:32 * (j + 1), j:j + 1], 1.0)
            blkT = cpool.tile([4, 128], F32)
            psbt = pp_tr.tile([128, 128], F32, tag="ptr", name="psbt")
            nc.tensor.transpose(out=psbt[0:4, :], in_=blkmask[:], identity=ident[:])
            nc.vector.tensor_copy(out=blkT[:], in_=psbt[0:4, :])

            iota_row = cpool.tile([1, 32], F32)
            nc.sync.dma_start(out=iota_row[:], in_=pr["iota32"][:])
            ps_small = ptile()
            nc.tensor.matmul(out=ps_small[:, 0:32], lhsT=ones_r[:], rhs=iota_row[:],
                             start=True, stop=True)
            iota_t = cpool.tile([128, 32], F32)
            nc.scalar.copy(out=iota_t[:], in_=ps_small[:, 0:32])

            def bcast_scalar(name, src):
                t0 = cpool.tile([1, 1], F32, tag=f"{name}_r")
                nc.sync.dma_start(out=t0[:], in_=src[:])
                psb_ = ptile()
                nc.tensor.matmul(out=psb_[:, 0:1], lhsT=ones_r[:], rhs=t0[:],
                                 start=True, stop=True)
                t = cpool.tile([128, 1], F32, tag=f"{name}_b")
                nc.scalar.copy(out=t[:], in_=psb_[:, 0:1])
                return t

            b1b = bcast_scalar("b1", pr["sc_b1"])
            b2b = bcast_scalar("b2", pr["sc_b2"])
            db1b = bcast_scalar("db1", pr["dp_b1"])
            db2b = bcast_scalar("db2", pr["dp_b2"])

            P3 = cpool.tile([128, 2, 3], F32)
            for cc in range(2):
                nc.sync.dma_start(out=P3[:, cc, 0:1], in_=pr["sc_w1"][128 * cc:128 * (cc + 1), :])
                nc.sync.dma_start(out=P3[:, cc, 1:2], in_=pr["sc_w2"][128 * cc:128 * (cc + 1), :])
                nc.sync.dma_start(out=P3[:, cc, 2:3], in_=pr["out_w"][128 * cc:128 * (cc + 1), :])

            wpats = []
            for b in range(3):
                wp = cpool.tile([128, 16], F32, tag=f"wpat{b}")
                nc.vector.memset(wp[:], 0.0)
                pstart = 0
                while pstart < 128:
                    jj, c0 = divmod(128 * b + pstart, EC)
                    run = min(128 - pstart, EC - c0)
                    for w, dpw in ((0, pr["dp_w1"]), (1, pr["dp_w2"])):
                        nc.sync.dma_start(
                            out=wp[pstart:pstart + run, 2 * jj + w:2 * jj + w + 1],
                            in_=dpw[c0:c0 + run, :])
                    pstart += run
                wpats.append(wp)


            # ---------------- per-node tiles ----------------
            proj = npool.tile([128, NCOL, 3], F32)
            num_t = npool.tile([128, NCOL], F32)
            cnt_t = npool.tile([128, NCOL], F32)
            score1 = npool.tile([128, NCOL], F32)
            t1 = npool.tile([128, NCOL], F32)
            kept1 = npool.tile([128, NCOL], F32)
            xs1t = npool.tile([128, NCOL], F32)
            m_t = npool.tile([128, NCOL], F32)
            num2_t = npool.tile([128, NCOL], F32)
            cnt2_t = npool.tile([128, NCOL], F32)
            score2 = npool.tile([128, NCOL], F32)
            score2m = npool.tile([128, NCOL], F32)
            t2 = npool.tile([128, NCOL], F32)
            kept2 = npool.tile([128, NCOL], F32)
            negbig = npool.tile([128, NCOL], F32)
            nc.vector.memset(negbig[:], -1e30)
            ko = npool.tile([1, 2 * G], F32)
            ko2 = npool.tile([1, 2 * G], F32)
            numH = [npool.tile([128, NCOL], F32, tag=f"numH{h}", name=f"numH{h}") for h in range(2)]
            cntH = [npool.tile([128, NCOL], F32, tag=f"cntH{h}", name=f"cntH{h}") for h in range(2)]

            xf = x.rearrange("n c -> (n c)")
            eaf = ea.rearrange("e c -> (e c)")

            for _rep in range(reps):
              with (tc.For_i(0, loop, 1) if loop > 1 else nullcontext()):
                # ---------------- x projection ----------------
                # node n -> partition (n//32)%128, col 32*(n//4096) + n%32
                for k in range(2):
                    for lh in range(4):
                        xg = wpool.tile([128, 8, C], F32, tag="xg")
                        src = bass.AP(xf.tensor,
                                      xf.offset + C * (4096 * k + 8 * lh),
                                      [[32 * C, 128], [C, 8], [1, C]])
                        nc.sync.dma_start(out=xg[:], in_=src)
                        psx = ptile()
                        for li in range(8):
                            for cc in range(2):
                                pst = ptile()
                                nc.tensor.transpose(
                                    out=pst[:], in_=xg[:, li, 128 * cc:128 * (cc + 1)],
                                    identity=ident[:])
                                xT = spool.tile([128, 128], F32, tag="xT")
                                if (li + cc) % 2 == 0:
                                    nc.scalar.copy(out=xT[:], in_=pst[:])
                                else:
                                    nc.vector.tensor_copy(out=xT[:], in_=pst[:])
                                nc.tensor.matmul(out=psx[:, 3 * li:3 * (li + 1)],
                                                 lhsT=xT[:], rhs=P3[:, cc, :],
                                                 start=(cc == 0), stop=(cc == 1))
                        c0 = 32 * k + 8 * lh
                        nc.vector.tensor_copy(
                            out=proj[:, c0:c0 + 8, :].rearrange("p a b -> p (a b)"),
                            in_=psx[:, 0:24])

                nc.vector.tensor_scalar(out=xs1t[:], in0=proj[:, :, 0], scalar1=b1b[:, 0:1],
                                        scalar2=None, op0=AL.add)

                # ---------------- ea projection ----------------
                Wboth = epool.tile([128, SLOTS, 2], F32)
                for t8 in range(0, E // 1024, 4):
                    reg8 = wpool.tile([128, 4, 384], F32, tag="eareg")
                    src = bass.AP(eaf.tensor, eaf.offset + 1024 * t8 * EC,
                                  [[8 * EC, 128], [1024 * EC, 4], [1, 384]])
                    nc.sync.dma_start(out=reg8[:], in_=src)
                    psw = ptile()
                    for ti in range(4):
                        for b in range(3):
                            pst = ptile()
                            nc.tensor.transpose(
                                out=pst[:],
                                in_=reg8[:, ti, 128 * b:128 * (b + 1)],
                                identity=ident[:])
                            tsb = spool.tile([128, 128], F32, tag="tsb")
                            if (ti + b) % 2 == 0:
                                nc.scalar.copy(out=tsb[:], in_=pst[:])
                            else:
                                nc.vector.tensor_copy(out=tsb[:], in_=pst[:])
                            nc.tensor.matmul(out=psw[:, 16 * ti:16 * (ti + 1)],
                                             lhsT=tsb[:], rhs=wpats[b][:],
                                             start=(b == 0), stop=(b == 2))
                    nc.scalar.copy(
                        out=Wboth[:, 8 * t8:8 * (t8 + 4), :].rearrange("p a b -> p (a b)"),
                        in_=psw[:, 0:64])

                # per-half W tiles: col 64j+u <- slot 128j + 64hh + u
                Wh = []
                for w, db in ((0, db1b), (1, db2b)):
                    for hh in range(2):
                        wt = epool.tile([128, HS], F32, tag=f"W{w}h{hh}")
                        nc.vector.tensor_scalar(
                            out=wt[:],
                            in0=_ap(Wboth, 2 * 64 * hh + w, [[256, 8], [2, 64]]),
                            scalar1=db[:, 0:1], scalar2=None, op0=AL.add)
                        Wh.append(wt)
                W1h, W2h = Wh[0:2], Wh[2:4]

                # ---------------- dst hi/lo + gather idx + cnt1 (host) ----------------
                hi_f = epool.tile([128, SLOTS], F32)
                lo_f = epool.tile([128, SLOTS], F32)
                nc.sync.dma_start(out=hi_f[:], in_=hi_d[:])
                nc.sync.dma_start(out=lo_f[:], in_=lo_d[:])
                gidx16 = epool.tile([128, SLOTS], I16)
                nc.sync.dma_start(out=gidx16[:], in_=gidx[:])
                nc.sync.dma_start(out=cnt_t[:], in_=cnt1_d[:])

                table = epool.tile([128, NN], F32)
                nc.vector.memset(table[:], 0.0)
                gouts = [epool.tile([128, 4096], F32, tag=f"gout{i}", name=f"gout{i}") for i in range(2)]
                compact_h = [epool.tile([128, HS], F32, tag=f"cmp{h}", name=f"cmph{h}") for h in range(2)]

                def build_table(src_tile):
                    pst = ptile()
                    nc.tensor.transpose(out=pst[:NCOL, :], in_=src_tile[:], identity=ident[:])
                    mT = wpool.tile([NCOL, 128], F32, tag="mT")
                    nc.vector.tensor_copy(out=mT[:], in_=pst[:NCOL, :])
                    nc.sync.dma_start(out=bounce.rearrange("(a b) -> a b", a=NCOL), in_=mT[:])
                    for k in range(8):
                        nc.sync.dma_start(out=table[16 * k:16 * k + 1, :],
                                          in_=bounce[None, :])

                def gather_compact():
                    if no_gather:
                        for h in range(2):
                            nc.vector.memset(compact_h[h][:], 0.0)
                        return
                    for q in range(4):
                        go = gouts[q % 2]
                        nc.gpsimd.ap_gather(go[:], table[:],
                                            gidx16[:, 256 * q:256 * (q + 1)],
                                            channels=128, num_elems=NN, d=1,
                                            num_idxs=4096)
                        hh, qq = q // 2, q % 2
                        for bp in range(32):
                            pst = ptile()
                            nc.tensor.transpose(out=pst[:],
                                                in_=go[:, 128 * bp:128 * (bp + 1)],
                                                identity=ident[:])
                            csrc = _ap(pst, 0, [[16, 8]])
                            cdst = _ap(compact_h[hh], 32 * qq + bp, [[64, 8]])
                            nc.scalar.copy(out=cdst, in_=csrc)

                def bilinear(msg_h, cnt_src_h, num_out, cnt_out):
                    """Segment sum into v2 node layout, half-split over hh so each
                    half only depends on gather calls 2hh, 2hh+1."""
                    W = 32 if cnt_src_h is None else 64
                    for hh in range(2):
                        for bb in range(2):
                            sb4n = wpool.tile([32, 4, 32], F32, tag="sb4n")
                            if W == 64:
                                sb4c = wpool.tile([32, 4, 32], F32, tag="sb4c")
                            for aa in range(4):
                                g = 4 * bb + aa
                                s0 = 128 * g + 64 * hh
                                c0 = 64 * g
                                psb = ppsb.tile([64, 32], F32, tag="psb")
                                TH = bpool.tile([128, 64, 64], F32, tag="TH")
                                L = bpool.tile([128, 64, 32], F32, tag="L")
                                lo_ap = _ap(lo_f, s0, [[1, 64], [0, 32]])
                                hi_ap = _ap(hi_f, s0, [[1, 64], [0, 32]])
                                io_ap = _ap(iota_t, 0, [[0, 64], [1, 32]])
                                nc.vector.tensor_tensor(out=L[:], in0=lo_ap, in1=io_ap,
                                                        op=AL.is_equal)
                                nc.vector.tensor_tensor(out=TH[:, :, 32:64], in0=hi_ap,
                                                        in1=io_ap, op=AL.is_equal)
                                msg_ap = _ap(msg_h[hh], c0, [[1, 64], [0, 32]])
                                nc.vector.tensor_tensor(out=TH[:, :, 0:32],
                                                        in0=TH[:, :, 32:64], in1=msg_ap,
                                                        op=AL.mult)
                                if cnt_src_h is not None:
                                    cs_ap = _ap(cnt_src_h[hh], c0, [[1, 64], [0, 32]])
                                    nc.vector.tensor_tensor(out=TH[:, :, 32:64],
                                                            in0=TH[:, :, 32:64], in1=cs_ap,
                                                            op=AL.mult)
                                for si in range(64):
                                    nc.tensor.matmul(out=psb[:W, :],
                                                     lhsT=TH[:, si, 0:W],
                                                     rhs=L[:, si, :],
                                                     start=(si == 0),
                                                     stop=(si == 63))
                                sb1 = wpool.tile([64, 32], F32, tag="sb1")
                                if (aa % 2) == 0:
                                    nc.vector.tensor_copy(out=sb1[:W, :], in_=psb[:W, :])
                                else:
                                    nc.scalar.copy(out=sb1[:W, :], in_=psb[:W, :])
                                fx = ptile()
                                pst2 = fx[0:32, :]
                                nc.tensor.transpose(out=pst2[:, 0:W], in_=sb1[:W, :],
                                                    identity=ident[:W, :W])
                                if (aa % 2) == 0:
                                    nc.scalar.copy(out=sb4n[:, aa, :], in_=pst2[:, 0:32])
                                else:
                                    nc.vector.tensor_copy(out=sb4n[:, aa, :], in_=pst2[:, 0:32])
                                if W == 64:
                                    if (aa % 2) == 0:
                                        nc.vector.tensor_copy(out=sb4c[:, aa, :],
                                                              in_=pst2[:, 32:64])
                                    else:
                                        nc.scalar.copy(out=sb4c[:, aa, :], in_=pst2[:, 32:64])
                            fxn = ptile()
                            nc.tensor.transpose(out=fxn[:, 0:32],
                                                in_=sb4n[:].rearrange("p a b -> p (a b)"),
                                                identity=ident[:32, :32])
                            nc.scalar.copy(out=num_out[hh][:, 32 * bb:32 * (bb + 1)],
                                           in_=fxn[:, 0:32])
                            if W == 64:
                                fxc = ptile()
                                nc.tensor.transpose(out=fxc[:, 0:32],
                                                    in_=sb4c[:].rearrange("p a b -> p (a b)"),
                                                    identity=ident[:32, :32])
                                nc.vector.tensor_copy(
                                    out=cnt_out[hh][:, 32 * bb:32 * (bb + 1)],
                                    in_=fxc[:, 0:32])

                def mean_guard(numt, cntt, out):
                    cm = wpool.tile([128, NCOL], F32, tag="cm")
                    nc.vector.tensor_scalar_max(cm[:], cntt[:], 1.0)
                    dv = wpool.tile([128, NCOL], F32, tag="dv")
                    nc.vector.reciprocal(out=cm[:], in_=cm[:])
                    nc.vector.tensor_tensor(out=dv[:], in0=numt[:], in1=cm[:], op=AL.mult)
                    mk = wpool.tile([128, NCOL], I8, tag="mk")
                    nc.vector.tensor_scalar(out=mk[:], in0=cntt[:], scalar1=0.0, scalar2=None,
                                            op0=AL.is_gt)
                    zz = wpool.tile([128, NCOL], F32, tag="zz")
                    nc.vector.memset(zz[:], 0.0)
                    nc.vector.select(out=out[:], mask=mk[:], on_true=dv[:], on_false=zz[:])

                def thresholds_mask(sc_cmp, sc_mm, kval, mask_out):
                    """mask_out = (sc_cmp >= tau_g) where tau_g is found by 4-ary
                    bisection so that each graph keeps exactly kval nodes.
                    sc_mm provides finite per-graph min/max to bracket the search."""
                    if no_kth:
                        nc.vector.memset(mask_out[:], 0.0)
                        return
                    # per-graph min/max of sc_mm -> [4 (a), 2 (b)] tiles
                    red = wpool.tile([128, 4], F32, tag="bsred")
                    nc.vector.tensor_reduce(out=red[:, 0:2],
                                            in_=sc_mm[:].rearrange("p (b c) -> p b c", b=2),
                                            axis=mybir.AxisListType.X, op=AL.min)
                    nc.vector.tensor_reduce(out=red[:, 2:4],
                                            in_=sc_mm[:].rearrange("p (b c) -> p b c", b=2),
                                            axis=mybir.AxisListType.X, op=AL.max)
                    # negate the max half so a single min-reduce works after transpose
                    nc.vector.tensor_scalar(out=red[:, 2:4], in0=red[:, 2:4], scalar1=-1.0,
                                            scalar2=None, op0=AL.mult)
                    prt = ptile()
                    nc.tensor.transpose(out=prt[0:4, :], in_=red[:], identity=ident[:])
                    redT = wpool.tile([4, 4, 32], F32, tag="bsredT")
                    nc.vector.tensor_copy(out=redT[:].rearrange("p a b -> p (a b)"),
                                          in_=prt[0:4, :])
                    mm4 = wpool.tile([4, 4], F32, tag="bsmm4")
                    nc.vector.tensor_reduce(out=mm4[:], in_=redT[:],
                                            axis=mybir.AxisListType.X, op=AL.min)
                    # mm4[c, a]: rows = (min b0, min b1, -max b0, -max b1) -> transpose
                    prt2 = ptile()
                    nc.tensor.transpose(out=prt2[0:4, 0:4], in_=mm4[:],
                                        identity=ident[:4, :4])
                    lo = wpool.tile([4, 2], F32, tag="bslo")
                    hi = wpool.tile([4, 2], F32, tag="bshi")
                    nc.vector.tensor_scalar(out=lo[:], in0=prt2[0:4, 0:2], scalar1=-1.0,
                                            scalar2=None, op0=AL.add)
                    nc.vector.tensor_scalar(out=hi[:], in0=prt2[0:4, 2:4], scalar1=-1.0,
                                            scalar2=1.0, op0=AL.mult, op1=AL.add)
                    mids = wpool.tile([4, 3, 2], F32, tag="bsmids")
                    span = wpool.tile([4, 2], F32, tag="bsspan")
                    cmp = wpool.tile([128, 3, 2, 32], F32, tag="bscmp")
                    redc = wpool.tile([128, 6], F32, tag="bsredc")
                    cntS = wpool.tile([4, 6], F32, tag="bscnt")
                    cond = wpool.tile([4, 6], I8, tag="bscond")
                    condN = wpool.tile([4, 6], I8, tag="bscondN")
                    taub = wpool.tile([128, 6], F32, tag="bstaub")
                    if debug and kval == 512.0:
                        nc.sync.dma_start(out=dbg["d_bs"][:, 0:2], in_=lo[:])
                        nc.sync.dma_start(out=dbg["d_bs"][:, 2:4], in_=hi[:])
                    for it in range(16):
                        # mids j = lo + (j+1)/4*(hi-lo), j=0,1,2
                        nc.vector.tensor_tensor(out=span[:], in0=hi[:], in1=lo[:],
                                                op=AL.subtract)
                        for j in range(3):
                            nc.vector.scalar_tensor_tensor(
                                out=mids[:, j, :], in0=span[:], scalar=0.25 * (j + 1),
                                in1=lo[:], op0=AL.mult, op1=AL.add)
                        ptau = ptile()
                        nc.tensor.matmul(out=ptau[0:128, 0:6], lhsT=blkT[:],
                                         rhs=mids[:].rearrange("p a b -> p (a b)"),
                                         start=True, stop=True)
                        nc.scalar.copy(out=taub[:], in_=ptau[0:128, 0:6])
                        nc.vector.tensor_tensor(
                            out=cmp[:].rearrange("p a b c -> p (a b c)"),
                            in0=_ap(sc_cmp, 0, [[0, 3], [32, 2], [1, 32]]),
                            in1=_ap(taub, 0, [[2, 3], [1, 2], [0, 32]]),
                            op=AL.is_ge)
                        nc.vector.tensor_reduce(out=redc[:].rearrange("p (a b) -> p a b", a=6),
                                                in_=cmp[:].rearrange("p a b c -> p (a b) c"),
                                                axis=mybir.AxisListType.X, op=AL.add)
                        pcnt = ptile()
                        nc.tensor.matmul(out=pcnt[0:4, 0:6], lhsT=blkmask[:], rhs=redc[:],
                                         start=True, stop=True)
                        nc.scalar.copy(out=cntS[:], in_=pcnt[0:4, 0:6])
                        nc.vector.tensor_scalar(out=cond[:], in0=cntS[:], scalar1=kval,
                                                scalar2=None, op0=AL.is_ge)
                        nc.vector.tensor_scalar(out=condN[:], in0=cntS[:], scalar1=kval,
                                                scalar2=None, op0=AL.is_lt)
                        # lo' = highest mid with cnt>=K (else lo); hi' = lowest mid with cnt<K (else hi)
                        for j in range(3):
                            nc.vector.select(out=lo[:], mask=cond[:, 2 * j:2 * (j + 1)],
                                             on_true=mids[:, j, :], on_false=lo[:])
                        for j in (2, 1, 0):
                            nc.vector.select(out=hi[:], mask=condN[:, 2 * j:2 * (j + 1)],
                                             on_true=mids[:, j, :], on_false=hi[:])
                        if debug and kval == 512.0 and it == 0:
                            nc.sync.dma_start(out=dbg["d_bs"][:, 4:10], in_=cntS[:])
                            nc.sync.dma_start(out=dbg["d_bs"][:, 10:16],
                                              in_=mids[:].rearrange("p a b -> p (a b)"))
                            nc.sync.dma_start(out=dbg["d_bs"][:, 16:18], in_=lo[:])
                            nc.sync.dma_start(out=dbg["d_bs"][:, 18:20], in_=hi[:])
                    if debug and kval == 512.0:
                        nc.sync.dma_start(out=dbg["d_bs"][:, 20:22], in_=lo[:])
                    ptau = ptile()
                    nc.tensor.matmul(out=ptau[0:128, 0:2], lhsT=blkT[:], rhs=lo[:],
                                     start=True, stop=True)
                    taulo = wpool.tile([128, 2], F32, tag="bstaulo")
                    nc.scalar.copy(out=taulo[:], in_=ptau[0:128, 0:2])
                    nc.vector.tensor_tensor(out=mask_out[:], in0=sc_tile_ap(sc_cmp),
                                            in1=_ap(taulo, 0, [[1, 2], [0, 32]]),
                                            op=AL.is_ge)

                def sc_tile_ap(t):
                    return t[:]

                # ================= LAYER 1 =================
                if stage < 2:
                    continue
                build_table(xs1t)
                gather_compact()
                if stage < 3:
                    continue
                msg_h = []
                for hh in range(2):
                    mh = epool.tile([128, HS], F32, tag=f"msg{hh}")
                    nc.vector.tensor_tensor(out=mh[:], in0=compact_h[hh][:],
                                            in1=W1h[hh][:], op=AL.mult)
                    msg_h.append(mh)
                bilinear(msg_h, None, numH, None)
                nc.vector.tensor_tensor(out=num_t[:], in0=numH[0][:], in1=numH[1][:],
                                        op=AL.add)
                if debug:
                    for hh in range(2):
                        nc.sync.dma_start(
                            out=_ap(dbg["d_compact1"], 64 * hh, [[128, 8], [1, 64]]),
                            in_=compact_h[hh][:].rearrange("p (a b) -> p a b", a=8))
                    nc.sync.dma_start(out=dbg["d_num"][:], in_=num_t[:])
                    nc.sync.dma_start(out=dbg["d_cnt"][:], in_=cnt_t[:])
                if stage < 4:
                    continue
                mean_guard(num_t, cnt_t, score1)
                thresholds_mask(score1, score1, 512.0, kept1)
                nc.scalar.activation(out=t1[:], in_=score1[:], func=ACTF.Tanh)
                nc.vector.tensor_tensor(out=m_t[:], in0=proj[:, :, 1], in1=t1[:], op=AL.mult)
                nc.vector.tensor_scalar(out=m_t[:], in0=m_t[:], scalar1=b2b[:, 0:1],
                                        scalar2=None, op0=AL.add)
                nc.vector.tensor_tensor(out=m_t[:], in0=m_t[:], in1=kept1[:], op=AL.mult)

                # ================= LAYER 2 =================
                if stage < 5:
                    continue
                build_table(m_t)
                gather_compact()
                msg2_h = []
                ksrc_h = []
                for hh in range(2):
                    mh = epool.tile([128, HS], F32, tag=f"msg{hh}")
                    nc.vector.tensor_tensor(out=mh[:], in0=compact_h[hh][:],
                                            in1=W2h[hh][:], op=AL.mult)
                    msg2_h.append(mh)
                    kh = epool.tile([128, HS], F32, tag=f"ksrc{hh}")
                    nc.vector.tensor_scalar(out=kh[:], in0=compact_h[hh][:], scalar1=0.0,
                                            scalar2=None, op0=AL.not_equal)
                    ksrc_h.append(kh)
                if stage < 6:
                    continue
                bilinear(msg2_h, ksrc_h, numH, cntH)
                nc.vector.tensor_tensor(out=num2_t[:], in0=numH[0][:], in1=numH[1][:],
                                        op=AL.add)
                nc.vector.tensor_tensor(out=cnt2_t[:], in0=cntH[0][:], in1=cntH[1][:],
                                        op=AL.add)
                if debug:
                    for hh in range(2):
                        nc.sync.dma_start(
                            out=_ap(dbg["d_compact2"], 64 * hh, [[128, 8], [1, 64]]),
                            in_=compact_h[hh][:].rearrange("p (a b) -> p a b", a=8))
                    nc.sync.dma_start(out=dbg["d_num2"][:], in_=num2_t[:])
                    nc.sync.dma_start(out=dbg["d_cnt2"][:], in_=cnt2_t[:])
                mean_guard(num2_t, cnt2_t, score2)
                kept1_i8 = wpool.tile([128, NCOL], I8, tag="k1i8")
                nc.vector.tensor_copy(out=kept1_i8[:], in_=kept1[:])
                nc.vector.select(out=score2m[:], mask=kept1_i8[:], on_true=score2[:],
                                 on_false=negbig[:])
                thresholds_mask(score2m, score2, 256.0, kept2)
                nc.vector.tensor_tensor(out=kept2[:], in0=kept2[:], in1=kept1[:], op=AL.mult)
                nc.scalar.activation(out=t2[:], in_=score2[:], func=ACTF.Tanh)

                # ================= FINAL =================
                acc = wpool.tile([128, NCOL], F32, tag="acc")
                nc.vector.tensor_tensor(out=acc[:], in0=kept2[:], in1=t2[:], op=AL.mult)
                nc.vector.tensor_scalar(out=acc[:], in0=acc[:], scalar1=1.0, scalar2=None,
                                        op0=AL.add)
                nc.vector.tensor_tensor(out=acc[:], in0=acc[:], in1=t1[:], op=AL.mult)
                nc.vector.tensor_tensor(out=acc[:], in0=acc[:], in1=kept1[:], op=AL.mult)
                nc.vector.tensor_scalar(out=acc[:], in0=acc[:], scalar1=1.0, scalar2=None,
                                        op0=AL.add)
                nc.vector.tensor_tensor(out=acc[:], in0=acc[:], in1=proj[:, :, 2],
                                        op=AL.mult)
                part = wpool.tile([128, 2], F32, tag="part")
                nc.vector.tensor_reduce(out=part[:],
                                        in_=acc[:].rearrange("p (b c) -> p b c", b=2),
                                        axis=mybir.AxisListType.X, op=AL.add)
                psS = ptile()
                nc.tensor.matmul(out=psS[:4, 0:2], lhsT=blkmask[:], rhs=part[:],
                                 start=True, stop=True)
                outb_r = cpool.tile([4, 1], F32, tag="outb")
                ob = pr["out_b"][:]
                nc.sync.dma_start(out=outb_r[:],
                                  in_=bass.AP(ob.tensor, ob.offset, [[0, 4], [1, 1]]))
                sres = wpool.tile([4, 2], F32, tag="sres")
                nc.scalar.activation(out=sres[:], in_=psS[:4, 0:2], func=ACTF.Sigmoid,
                                     bias=outb_r[:, 0:1])
                nc.sync.dma_start(out=outp.rearrange("(b a) o -> a (b o)", a=4),
                                  in_=sres[:])
                if debug:
                    nc.sync.dma_start(out=dbg["d_proj"][:],
                                      in_=proj[:].rearrange("p a b -> p (a b)"))
                    for nm, tt in (("d_score1", score1), ("d_kept1", kept1),
                                   ("d_m", m_t), ("d_score2", score2),
                                   ("d_kept2", kept2)):
                        nc.sync.dma_start(out=dbg[nm][:], in_=tt[:])
                    nc.sync.dma_start(out=dbg["d_w"][:, 0:SLOTS], in_=lo_f[:])
                    nc.sync.dma_start(out=dbg["d_w"][:, SLOTS:2 * SLOTS], in_=hi_f[:])

    nc.finalize()
    return nc


# ---------------------------------------------------------------------------
_E_OF_SLOT = None
_GIDX_EDGE = None


def _slot_maps():
    global _E_OF_SLOT, _GIDX_EDGE
    if _E_OF_SLOT is None:
        p = np.arange(128)[:, None]
        s = np.arange(SLOTS)[None, :]
        _E_OF_SLOT = 1024 * (s // 8) + 8 * p + (s % 8)
        j = np.arange(EPG)
        b = j // 128
        pp_ = j % 128
        _GIDX_EDGE = 1024 * (b // 8) + 8 * pp_ + (b % 8)
    return _E_OF_SLOT, _GIDX_EDGE


def _table_pos(n):
    """node id -> bounce-table position: 128*C(n) + q(n)."""
    return 128 * (32 * (n // 4096) + n % 32) + (n // 32) % 128


def make_core_inputs(inputs, core):
    e_of_slot, gidx_edge = _slot_maps()
    n0 = core * NN
    e0 = core * E
    src = np.asarray(inputs["edge_index"][0, e0:e0 + E], np.int64) - n0
    dst = np.asarray(inputs["edge_index"][1, e0:e0 + E], np.int64) - n0
    tpos = _table_pos(src)
    gi = np.empty((128, SLOTS), np.int16)
    jj = np.arange(EPG)
    for k in range(8):
        gi[16 * k + jj % 16, jj // 16] = tpos[EPG * k + gidx_edge]
    dst_slot = dst[e_of_slot]
    dloc = dst_slot % NPG
    deg = np.bincount(dst, minlength=NN).astype(np.float32)
    qq = np.arange(128)[:, None]
    cc_ = np.arange(NCOL)[None, :]
    n_of_qc = 4096 * (cc_ // 32) + 32 * qq + (cc_ % 32)
    d = dict(
        x=np.ascontiguousarray(inputs["x"][n0:n0 + NN], dtype=np.float32),
        ea=np.ascontiguousarray(inputs["edge_attr"][e0:e0 + E], dtype=np.float32),
        lo_d=(dloc % 32).astype(np.float32),
        hi_d=(dloc // 32).astype(np.float32),
        gidx=gi,
        cnt1_d=deg[n_of_qc],
        iota32=np.arange(32, dtype=np.float32).reshape(1, 32),
    )
    for nm, shp in (("dp_w1", (EC, 1)), ("dp_b1", (1, 1)), ("sc_w1", (C, 1)),
                    ("sc_b1", (1, 1)), ("dp_w2", (EC, 1)), ("dp_b2", (1, 1)),
                    ("sc_w2", (C, 1)), ("sc_b2", (1, 1)), ("out_w", (C, 1)),
                    ("out_b", (1, 1))):
        d[nm] = np.asarray(inputs[nm], np.float32).reshape(shp)
    return d


_NC_CACHE = None


def kernel(**inputs):
    global _NC_CACHE
    if _NC_CACHE is None:
        _NC_CACHE = build_program()
    in_maps = [make_core_inputs(inputs, c) for c in range(8)]
    res = run_bass_kernel_spmd(_NC_CACHE, in_maps, list(range(8)))
    return np.concatenate([res.results[c]["out"] for c in range(8)], axis=0)
